# revision 20
# baseline (speedup 1.0000x reference)
"""Bass/Trainium2 kernel for nn_MaskedLoss (MSE with bbox-ROI weighting).

Self-contained: hardcodes shapes (4,1,160,160,160) f32/i32, shards across
8 NeuronCores as (batch item, D-half) pairs, and combines per-core
partial sums on the host.

v7 — collective-free design (~12.3 MB streamed per core):
  - y_pred/y_true cast to bf16 on host (loss is a 16.7M-element mean;
    input rounding is ~1e-5 on the result). Mask cast to fp8e4m3
    (exact for 0/1 values).
  - Each core loads the FULL mask of its batch item (both D-halves) and
    computes the bbox locally — no AllReduce, no cross-core skew, no
    dynamic-index extracts. The two cores of a pair compute identical
    boxes by construction.
  - Mask column-any on PE (fp8 ones-matmuls, 100-op PSUM accumulation).
  - Mask row sums split DVE (slabs 0-6, reduce-X) / ACT (slabs 7-9,
    per-row accum) to fill both engines' idle mask-phase windows.
  - d/h/w extrema via static coordinate tiles (d(row), h(row) are
    compile-time functions of the layout): max-reduce of
    gt_rows * (BIG +- coord), one partition all-reduce.
  - The w-box becomes a 0/1 weight vector (integer comparisons absorb
    the reference's floor()), applied as sq *= w01 followed by per-row
    reduce; the d/h-box and has_fg fold into per-row weights applied to
    those row sums at the end. Box bounds reproduce the reference's
    float32 arithmetic exactly (k >= floor(x) <=> k > x-1 for integer k).
"""

import os
import sys

import numpy as np

sys.path.insert(0, "/opt/trn_rl_repo")

B = 4                        # batch items
DS, HS, WS = 160, 160, 160   # spatial dims
HALF_D = DS // 2             # 80 d-slices per core
R = HALF_D * HS              # 12800 rows (d,h) per core (y data)
RF = DS * HS                 # 25600 rows: full-item mask
KJ = 4                       # rows per partition line
NT = R // (128 * KJ)         # 25 y-tiles per tensor per core
GT = 5                       # tiles per DMA/compute group
NG = NT // GT                # 5 y groups
GF = GT * KJ * WS            # 3200 free elems per group
GV = GT * KJ                 # 20 rows per partition line per group
NS = RF // (128 * KJ * GT)   # 10 mask slabs
N_CORES = 8
BIG = 1.0e6
W_OUT2 = 0.01                # W_OUT ** 2
EXPAND = 1.2
N_DVE_SLABS = 7              # mask rowsum slabs on DVE; rest ACT

_CACHE: dict = {}


def _build_nc():
    from concourse import bacc, bass, bass_isa, tile
    import concourse.mybir as mybir

    f32 = mybir.dt.float32
    bf16 = mybir.dt.bfloat16
    fp8 = mybir.dt.float8e4
    i32 = mybir.dt.int32
    AX = mybir.AxisListType
    OP = mybir.AluOpType
    AF = mybir.ActivationFunctionType
    RO = bass_isa.ReduceOp

    nc = bacc.Bacc(
        "TRN2", target_bir_lowering=False, debug=False, num_devices=N_CORES
    )

    yp = nc.dram_tensor("yp", [R, WS], bf16, kind="ExternalInput")
    yt = nc.dram_tensor("yt", [R, WS], bf16, kind="ExternalInput")
    mk = nc.dram_tensor("mk", [RF, WS], fp8, kind="ExternalInput")
    meta = nc.dram_tensor("meta", [1], f32, kind="ExternalInput")
    out = nc.dram_tensor("out", [2], f32, kind="ExternalOutput")

    ypv = yp.ap().rearrange("(g u p j) w -> g p u j w", p=128, j=KJ, u=GT)
    ytv = yt.ap().rearrange("(g u p j) w -> g p u j w", p=128, j=KJ, u=GT)
    mkv = mk.ap().rearrange("(s u p j) w -> s p u j w", p=128, j=KJ, u=GT)

    with tile.TileContext(nc) as tc:
        with (
            tc.tile_pool(name="persist", bufs=1) as pp,
            tc.tile_pool(name="pp2", bufs=3) as ppool,
            tc.tile_pool(name="tp2", bufs=3) as tpool,
            tc.tile_pool(name="psp", bufs=1,
                         space=bass.MemorySpace.PSUM) as pspool,
            tc.tile_pool(name="sqp", bufs=5) as sqpool,
            tc.tile_pool(name="asc", bufs=2) as ascratch,
        ):
            from concourse.tile_rust import add_dep_helper

            # ---- setup: constants and static coordinate tiles ----
            iota_w = pp.tile([1, WS], i32, tag="iota_w")
            nc.gpsimd.iota(iota_w[:], pattern=[[1, WS]], base=0,
                           channel_multiplier=0)
            k160 = pp.tile([1, WS], f32, tag="k160")
            nc.vector.tensor_copy(out=k160[:], in_=iota_w[:])
            bmk = pp.tile([1, WS], f32, tag="bmk")
            nc.vector.tensor_scalar(out=bmk[:], in0=k160[:], scalar1=-1.0,
                                    scalar2=BIG, op0=OP.mult, op1=OP.add)
            kpb = pp.tile([1, WS], f32, tag="kpb")
            nc.vector.tensor_scalar(out=kpb[:], in0=k160[:], scalar1=BIG,
                                    scalar2=None, op0=OP.add)
            ones_f8 = pp.tile([128, 1], fp8, tag="ones_f8")
            nc.gpsimd.memset(ones_f8[:], 1.0)

            meta_s = pp.tile([1, 1], f32, tag="meta_s")
            nc.gpsimd.dma_start(
                out=meta_s[:], in_=meta.ap().rearrange("(p x) -> p x", p=1))
            meta_b = pp.tile([128, 1], f32, tag="meta_b")
            nc.gpsimd.partition_broadcast(meta_b[:], meta_s[:], channels=128)

            def coord_tiles(ncols, nt_pat, tagp):
                # r = 4p + 512t + j over (t,j); d = r//160, h = r%160
                io = pp.tile([128, ncols], i32, tag=f"io_{tagp}")
                nc.gpsimd.iota(io[:].rearrange("p (t j) -> p t j", j=KJ),
                               pattern=[[512, nt_pat], [1, KJ]], base=0,
                               channel_multiplier=4)
                rf_ = pp.tile([128, ncols], f32, tag=f"rf_{tagp}")
                nc.vector.tensor_copy(out=rf_[:], in_=io[:])
                x = pp.tile([128, ncols], f32, tag=f"x_{tagp}")
                nc.vector.tensor_scalar(out=x[:], in0=rf_[:],
                                        scalar1=1.0 / 160.0, scalar2=None,
                                        op0=OP.mult)
                di = pp.tile([128, ncols], i32, tag=f"di_{tagp}")
                nc.vector.tensor_copy(out=di[:], in_=x[:])
                df = pp.tile([128, ncols], f32, tag=f"df_{tagp}")
                nc.vector.tensor_copy(out=df[:], in_=di[:])
                co = pp.tile([128, ncols], f32, tag=f"co_{tagp}")
                nc.vector.tensor_tensor(out=co[:], in0=df[:], in1=x[:],
                                        op=OP.is_gt)
                dl = pp.tile([128, ncols], f32, tag=f"dl_{tagp}")
                nc.vector.tensor_tensor(out=dl[:], in0=df[:], in1=co[:],
                                        op=OP.subtract)
                hneg = pp.tile([128, ncols], f32, tag=f"hn_{tagp}")
                nc.vector.tensor_scalar(out=hneg[:], in0=dl[:],
                                        scalar1=-160.0, scalar2=None,
                                        op0=OP.mult)
                hl = pp.tile([128, ncols], f32, tag=f"hl_{tagp}")
                nc.vector.tensor_tensor(out=hl[:], in0=rf_[:], in1=hneg[:],
                                        op=OP.add)
                return dl, hl

            # own rows: for the in_dh weights (d needs the meta offset)
            d_own, h_own = coord_tiles(NT * KJ, NT, "own")
            d_gpc = pp.tile([128, NT * KJ], f32, tag="d_gpc")
            nc.vector.tensor_scalar(out=d_gpc[:], in0=d_own[:],
                                    scalar1=meta_b[:, 0:1], scalar2=None,
                                    op0=OP.add)
            # full-item rows: for the bbox extrema (global d, no meta)
            d_ful, h_ful = coord_tiles(RF // 128, RF // (128 * KJ), "ful")
            coefs = []
            for k, (base, sgn) in enumerate(((d_ful, -1.0), (d_ful, 1.0),
                                             (h_ful, -1.0), (h_ful, 1.0))):
                cf = pp.tile([128, RF // 128], f32, tag=f"cf_{k}")
                nc.vector.tensor_scalar(out=cf[:], in0=base[:], scalar1=sgn,
                                        scalar2=BIG, op0=OP.mult, op1=OP.add)
                coefs.append(cf)

            # ---------------- phase 1: full-mask projections ------------
            mkA = pp.tile([128, 5 * GF], fp8, tag="mkA")  # slabs 0,2,4,6,8
            mkB = pp.tile([128, 5 * GF], fp8, tag="mkB")  # slabs 1,3,5,7,9
            acc_r = pp.tile([128, RF // 128], f32, tag="acc_r")
            colps = pspool.tile([1, 2 * WS], f32, tag="colps")

            slab_ap = []
            mask_sync_last = None
            mask_scal_last = None
            for s in range(NS):
                dst = (mkA if s % 2 == 0 else mkB)
                dsl = dst[:, (s // 2) * GF : (s // 2 + 1) * GF]
                dma = (nc.sync if s % 2 == 0 else nc.scalar).dma_start(
                    out=dsl.rearrange("p (u j w) -> p u j w", u=GT, j=KJ),
                    in_=mkv[s])
                if s % 2 == 0:
                    mask_sync_last = dma
                else:
                    mask_scal_last = dma
                slab_ap.append(dsl)

            for s in range(NS):
                for c in range(GF // (2 * WS)):
                    nc.tensor.matmul(
                        colps[:], ones_f8[:],
                        slab_ap[s][:, c * 2 * WS : (c + 1) * 2 * WS],
                        start=(s == 0 and c == 0),
                        stop=(s == NS - 1 and c == GF // (2 * WS) - 1))
            # ACT takes rowsum slabs 7-9 (fills its idle mask window);
            # DVE slabs 0-6 are issued interleaved with the bulk loop below
            with nc.allow_low_precision("0/1 mask sums are exact"):
                for s in range(N_DVE_SLABS, NS):
                    for v in range(GV):
                        scv = ascratch.tile([128, WS], bf16, tag="scv")
                        nc.scalar.activation(
                            out=scv[:],
                            in_=slab_ap[s][:, v * WS : (v + 1) * WS],
                            func=AF.Copy,
                            accum_out=acc_r[:, s * GV + v : s * GV + v + 1])

            # ---------------- bbox extrema ----------------
            anyw2 = pp.tile([1, 2 * WS], f32, tag="anyw2")
            nc.scalar.activation(out=anyw2[:], in_=colps[:], func=AF.Copy)
            v_w = pp.tile([1, WS], f32, tag="v_w")
            nc.vector.tensor_tensor(out=v_w[:], in0=anyw2[:, 0:WS],
                                    in1=anyw2[:, WS : 2 * WS], op=OP.add)
            gt_w = pp.tile([1, WS], f32, tag="gt_w")
            nc.vector.tensor_scalar(out=gt_w[:], in0=v_w[:], scalar1=1.0,
                                    scalar2=None, op0=OP.min)
            ta_w = pp.tile([1, WS], f32, tag="ta_w")
            nc.vector.tensor_tensor(out=ta_w[:], in0=gt_w[:], in1=bmk[:],
                                    op=OP.mult)
            ra_w = pp.tile([1, 1], f32, tag="ra_w")
            nc.vector.tensor_reduce(out=ra_w[:], in_=ta_w[:], axis=AX.X,
                                    op=OP.max)
            tb_w = pp.tile([1, WS], f32, tag="tb_w")
            nc.vector.tensor_tensor(out=tb_w[:], in0=gt_w[:], in1=kpb[:],
                                    op=OP.mult)
            rb_w = pp.tile([1, 1], f32, tag="rb_w")
            nc.vector.tensor_reduce(out=rb_w[:], in_=tb_w[:], axis=AX.X,
                                    op=OP.max)
            hf_sum = pp.tile([1, 1], f32, tag="hf_sum")
            nc.vector.tensor_reduce(out=hf_sum[:], in_=gt_w[:], axis=AX.X,
                                    op=OP.add)
            hf_loc = pp.tile([1, 1], f32, tag="hf_loc")
            nc.vector.tensor_scalar(out=hf_loc[:], in0=hf_sum[:], scalar1=1.0,
                                    scalar2=None, op0=OP.min)

            slw = pp.tile([1, 2], f32, tag="slw")  # [-mn_w, mx_w]
            nc.vector.tensor_scalar(out=slw[:, 0:1], in0=ra_w[:],
                                    scalar1=-BIG, scalar2=None, op0=OP.add)
            nc.vector.tensor_scalar(out=slw[:, 1:2], in0=rb_w[:],
                                    scalar1=-BIG, scalar2=None, op0=OP.add)

            # ---------------- box bounds (compare form) ----------------
            def bounds(mn_neg, mx, tagp):
                # inputs: mn_neg = -mn (exact), mx; returns lo-1, hi-1
                mn = pp.tile([1, 1], f32, tag=f"mn_{tagp}")
                nc.vector.tensor_scalar(out=mn[:], in0=mn_neg, scalar1=-1.0,
                                        scalar2=None, op0=OP.mult)
                c2 = pp.tile([1, 1], f32, tag=f"c2_{tagp}")
                nc.vector.tensor_tensor(out=c2[:], in0=mn[:], in1=mx,
                                        op=OP.add)
                cC = pp.tile([1, 1], f32, tag=f"cC_{tagp}")
                nc.vector.tensor_scalar(out=cC[:], in0=c2[:], scalar1=0.5,
                                        scalar2=None, op0=OP.mult)
                em = pp.tile([1, 1], f32, tag=f"em_{tagp}")
                nc.vector.tensor_tensor(out=em[:], in0=mx, in1=mn[:],
                                        op=OP.subtract)
                nc.vector.tensor_scalar(out=em[:], in0=em[:], scalar1=1.0,
                                        scalar2=0.5, op0=OP.add, op1=OP.mult)
                eE = pp.tile([1, 1], f32, tag=f"eE_{tagp}")
                nc.vector.tensor_scalar(out=eE[:], in0=em[:], scalar1=EXPAND,
                                        scalar2=None, op0=OP.mult)
                lo = pp.tile([1, 1], f32, tag=f"lo_{tagp}")
                nc.vector.tensor_tensor(out=lo[:], in0=cC[:], in1=eE[:],
                                        op=OP.subtract)
                nc.vector.tensor_scalar(out=lo[:], in0=lo[:], scalar1=-1.0,
                                        scalar2=None, op0=OP.add)
                hi = pp.tile([1, 1], f32, tag=f"hi_{tagp}")
                nc.vector.tensor_tensor(out=hi[:], in0=cC[:], in1=eE[:],
                                        op=OP.add)
                nc.vector.tensor_scalar(out=hi[:], in0=hi[:], scalar1=-1.0,
                                        scalar2=float(WS - 2), op0=OP.add,
                                        op1=OP.min)
                return lo, hi


            lo_w, hi_w = bounds(slw[:, 0:1], slw[:, 1:2], "w")
            # w01: 0/1 weight row over w, with has_fg folded in
            in_w = pp.tile([1, WS], f32, tag="in_w")
            wk0 = pp.tile([1, WS], f32, tag="wk0")
            nc.vector.tensor_scalar(out=in_w[:], in0=k160[:],
                                    scalar1=lo_w[:], scalar2=None,
                                    op0=OP.is_gt)
            nc.vector.tensor_scalar(out=wk0[:], in0=k160[:],
                                    scalar1=hi_w[:], scalar2=None,
                                    op0=OP.is_le)
            nc.vector.tensor_tensor(out=in_w[:], in0=in_w[:], in1=wk0[:],
                                    op=OP.mult)
            nc.vector.tensor_scalar(out=in_w[:], in0=in_w[:],
                                    scalar1=hf_loc[:], scalar2=None,
                                    op0=OP.mult)
            in_w_bf = pp.tile([1, WS], bf16, tag="in_w_bf")
            with nc.allow_low_precision("0/1 weights exact in bf16"):
                nc.vector.tensor_copy(out=in_w_bf[:], in_=in_w[:])
            w01b = pp.tile([128, WS], bf16, tag="w01b")
            nc.gpsimd.partition_broadcast(w01b[:], in_w_bf[:], channels=128)
            w01rep = pp.tile([128, GF], bf16, tag="w01rep")
            w01_last = None
            with nc.allow_low_precision("0/1 weights exact in bf16"):
                for v in range(GV):
                    w01_last = nc.vector.tensor_copy(
                        out=w01rep[:, v * WS : (v + 1) * WS], in_=w01b[:])

            # ---------------- phase 2: weighted MSE sums ----------------
            lp = nc.allow_low_precision("bf16 stream; f32 accumulation")
            lp.__enter__()
            acc_tot = pp.tile([128, NG], f32, tag="acc_tot")
            wrow = pp.tile([128, NT * KJ], f32, tag="wrow")
            def dve_rowsum(s):
                with nc.allow_low_precision("0/1 mask sums are exact"):
                    nc.vector.tensor_reduce(
                        out=acc_r[:, s * GV : (s + 1) * GV],
                        in_=slab_ap[s].rearrange("p (v w) -> p v w", v=GV),
                        axis=AX.X, op=OP.add)

            dve_rowsum(0)
            dve_rowsum(1)
            for g in range(NG):
                for s2 in range(2 + g, min(2 + g + 1, N_DVE_SLABS)):
                    dve_rowsum(s2)
                p_g = ppool.tile([128, GF], bf16, tag="p_g")
                yp_dma = nc.sync.dma_start(
                    out=p_g[:].rearrange("p (u j w) -> p u j w", u=GT, j=KJ),
                    in_=ypv[g])
                t_g = tpool.tile([128, GF], bf16, tag="t_g")
                yt_dma = nc.scalar.dma_start(
                    out=t_g[:].rearrange("p (u j w) -> p u j w", u=GT, j=KJ),
                    in_=ytv[g])
                add_dep_helper(yp_dma.ins, mask_sync_last.ins, sync=False,
                               reason="mask first on sync queue")
                add_dep_helper(yp_dma.ins, mask_scal_last.ins, sync=True,
                               reason="mask first (cross queue)")
                add_dep_helper(yt_dma.ins, mask_scal_last.ins, sync=False,
                               reason="mask first on scalar queue")
                add_dep_helper(yt_dma.ins, mask_sync_last.ins, sync=True,
                               reason="mask first (cross queue)")
                sub_i = nc.vector.tensor_tensor(out=p_g[:], in0=p_g[:],
                                                in1=t_g[:], op=OP.subtract)
                sq_g = sqpool.tile([128, GF], bf16, tag="sq_g")
                nc.scalar.activation(
                    out=sq_g[:], in_=p_g[:], func=AF.Square,
                    accum_out=acc_tot[:, g : g + 1])
                nc.vector.tensor_tensor(out=sq_g[:], in0=sq_g[:],
                                        in1=w01rep[:], op=OP.mult)
                nc.vector.tensor_reduce(
                    out=wrow[:, g * GV : (g + 1) * GV],
                    in_=sq_g[:].rearrange("p (v w) -> p v w", v=GV),
                    axis=AX.X, op=OP.add)
            lp.__exit__(None, None, None)

            gt_r = pp.tile([128, RF // 128], f32, tag="gt_r")
            nc.vector.tensor_scalar(out=gt_r[:], in0=acc_r[:], scalar1=1.0,
                                    scalar2=None, op0=OP.min)
            scr = pp.tile([128, RF // 128], f32, tag="scr")
            p4 = pp.tile([128, 4], f32, tag="p4")
            for k in range(4):
                nc.vector.tensor_tensor(out=scr[:], in0=gt_r[:],
                                        in1=coefs[k][:], op=OP.mult)
                nc.vector.tensor_reduce(out=p4[:, k : k + 1], in_=scr[:],
                                        axis=AX.X, op=OP.max)
            p4r = pp.tile([128, 4], f32, tag="p4r")
            nc.gpsimd.partition_all_reduce(p4r[:], p4[:], channels=128,
                                           reduce_op=RO.max)
            sl4 = pp.tile([1, 4], f32, tag="sl4")  # [-mn_d, mx_d, -mn_h, mx_h]
            nc.vector.tensor_scalar(out=sl4[:], in0=p4r[0:1, :],
                                    scalar1=-BIG, scalar2=None, op0=OP.add)
            lo_d, hi_d = bounds(sl4[:, 0:1], sl4[:, 1:2], "d")
            lo_h, hi_h = bounds(sl4[:, 2:3], sl4[:, 3:4], "h")
            # in_dh: per-own-row 0/1 weight from d/h bounds
            b4 = pp.tile([1, 4], f32, tag="b4")
            for k, srcb in enumerate((lo_d, hi_d, lo_h, hi_h)):
                nc.vector.tensor_copy(out=b4[:, k : k + 1], in_=srcb[:])
            b4b = pp.tile([128, 4], f32, tag="b4b")
            nc.gpsimd.partition_broadcast(b4b[:], b4[:], channels=128)
            in_dh = pp.tile([128, NT * KJ], f32, tag="in_dh")
            wk1 = pp.tile([128, NT * KJ], f32, tag="wk1")
            nc.vector.tensor_scalar(out=in_dh[:], in0=d_gpc[:],
                                    scalar1=b4b[:, 0:1], scalar2=None,
                                    op0=OP.is_gt)
            nc.vector.tensor_scalar(out=wk1[:], in0=d_gpc[:],
                                    scalar1=b4b[:, 1:2], scalar2=None,
                                    op0=OP.is_le)
            nc.vector.tensor_tensor(out=in_dh[:], in0=in_dh[:], in1=wk1[:],
                                    op=OP.mult)
            nc.vector.tensor_scalar(out=wk1[:], in0=h_own[:],
                                    scalar1=b4b[:, 2:3], scalar2=None,
                                    op0=OP.is_gt)
            nc.vector.tensor_tensor(out=in_dh[:], in0=in_dh[:], in1=wk1[:],
                                    op=OP.mult)
            nc.vector.tensor_scalar(out=wk1[:], in0=h_own[:],
                                    scalar1=b4b[:, 3:4], scalar2=None,
                                    op0=OP.is_le)
            nc.vector.tensor_tensor(out=in_dh[:], in0=in_dh[:], in1=wk1[:],
                                    op=OP.mult)

            # ---------------- final reductions ----------------
            tot_col = pp.tile([128, 1], f32, tag="tot_col")
            nc.vector.tensor_reduce(out=tot_col[:], in_=acc_tot[:],
                                    axis=AX.X, op=OP.add)
            junk_a = pp.tile([128, NT * KJ], f32, tag="junk_a")
            nc.vector.tensor_tensor(out=junk_a[:], in0=wrow[:],
                                    in1=in_dh[:], op=OP.mult)
            box_col = pp.tile([128, 1], f32, tag="box_col")
            nc.vector.tensor_reduce(out=box_col[:], in_=junk_a[:], axis=AX.X,
                                    op=OP.add)
            tot_r = pp.tile([128, 1], f32, tag="tot_r")
            nc.gpsimd.partition_all_reduce(tot_r[:], tot_col[:], channels=128,
                                           reduce_op=RO.add)
            box_r = pp.tile([128, 1], f32, tag="box_r")
            nc.gpsimd.partition_all_reduce(box_r[:], box_col[:], channels=128,
                                           reduce_op=RO.add)
            res2 = pp.tile([1, 2], f32, tag="res2")
            nc.vector.tensor_copy(out=res2[:, 0:1], in_=tot_r[0:1, :])
            nc.vector.tensor_copy(out=res2[:, 1:2], in_=box_r[0:1, :])
            nc.gpsimd.dma_start(
                out=out.ap().rearrange("(p x) -> p x", p=1), in_=res2[:])

    nc.compile()
    return nc


def get_nc():
    if "nc" not in _CACHE:
        _CACHE["nc"] = _build_nc()
    return _CACHE["nc"]


def make_in_maps(y_pred, y_true, mask):
    import ml_dtypes

    y_pred = np.asarray(y_pred, dtype=np.float32).reshape(B, DS, HS, WS)
    y_true = np.asarray(y_true, dtype=np.float32).reshape(B, DS, HS, WS)
    mask = np.asarray(mask, dtype=np.int32).reshape(B, DS, HS, WS)
    y_pred = y_pred.astype(ml_dtypes.bfloat16)
    y_true = y_true.astype(ml_dtypes.bfloat16)
    mask_f8 = mask.astype(ml_dtypes.float8_e4m3)  # 0/1 values: exact
    in_maps = []
    for c in range(N_CORES):
        b, half = c // 2, c % 2
        sl = slice(half * HALF_D, (half + 1) * HALF_D)
        in_maps.append({
            "yp": np.ascontiguousarray(y_pred[b, sl]).reshape(R, WS),
            "yt": np.ascontiguousarray(y_true[b, sl]).reshape(R, WS),
            "mk": np.ascontiguousarray(mask_f8[b]).reshape(RF, WS),
            "meta": np.array([half * HALF_D], dtype=np.float32),
        })
    return in_maps


def combine(results):
    tot = 0.0
    box = 0.0
    for r in results:
        o = np.asarray(r["out"], dtype=np.float64).reshape(-1)
        tot += o[0]
        box += o[1]
    loss = (W_OUT2 * tot + (1.0 - W_OUT2) * box) / float(B * DS * HS * WS)
    return np.array(loss, dtype=np.float32)


def kernel(y_pred, y_true, mask):
    from concourse.bass_utils import run_bass_kernel_spmd

    nc = get_nc()
    in_maps = make_in_maps(y_pred, y_true, mask)
    trace = bool(int(os.environ.get("BASS_KERNEL_TRACE", "0")))
    kwargs = {}
    if trace:
        kwargs = dict(trace=True, trace_cores=[0])
    res = run_bass_kernel_spmd(
        nc, in_maps, core_ids=list(range(N_CORES)), **kwargs
    )
    _CACHE["last_results"] = res
    return combine(res.results)


# revision 23
# speedup vs baseline: 1.0075x; 1.0075x over previous
"""Bass/Trainium2 kernel for nn_MaskedLoss (MSE with bbox-ROI weighting).

Self-contained: hardcodes shapes (4,1,160,160,160) f32/i32, shards across
8 NeuronCores as (batch item, D-half) pairs, and combines per-core
partial sums on the host.

Final design — collective-free (~12.3 MB streamed per core):
  - y_pred/y_true cast to bf16 on host (loss is a 16.7M-element mean;
    input rounding is ~1e-5 on the result). Mask cast to fp8e4m3
    (exact for 0/1 values).
  - Each core loads the FULL mask of its batch item (both D-halves) and
    computes the bbox locally — no AllReduce, no cross-core skew, no
    dynamic-index extracts. The two cores of a pair compute identical
    boxes by construction.
  - Mask column-any on PE (fp8 ones-matmuls, 100-op PSUM accumulation).
  - Mask row sums split DVE (slabs 0-6, reduce-X, issue-interleaved
    with the bulk pipeline) / ACT (slabs 7-9, per-row accum).
  - d/h/w extrema via static coordinate tiles (d(row), h(row) are
    compile-time functions of the layout): max-reduce of
    gt_rows * (BIG +- coord), one partition all-reduce.
  - The w-box becomes a 0/1 weight vector (integer comparisons absorb
    the reference's floor()), applied as sq *= w01 followed by per-row
    reduce; the d/h-box and has_fg fold into per-row weights applied to
    those row sums at the end. Box bounds reproduce the reference's
    float32 arithmetic exactly (k >= floor(x) <=> k > x-1 for integer k).
"""

import os
import sys

import numpy as np

sys.path.insert(0, "/opt/trn_rl_repo")

B = 4                        # batch items
DS, HS, WS = 160, 160, 160   # spatial dims
HALF_D = DS // 2             # 80 d-slices per core
R = HALF_D * HS              # 12800 rows (d,h) per core (y data)
RF = DS * HS                 # 25600 rows: full-item mask
KJ = 4                       # rows per partition line
NT = R // (128 * KJ)         # 25 y-tiles per tensor per core
GT = 5                       # tiles per DMA/compute group
NG = NT // GT                # 5 y groups
GF = GT * KJ * WS            # 3200 free elems per group
GV = GT * KJ                 # 20 rows per partition line per group
NS = RF // (128 * KJ * GT)   # 10 mask slabs
N_CORES = 8
BIG = 1.0e6
W_OUT2 = 0.01                # W_OUT ** 2
EXPAND = 1.2
N_DVE_SLABS = 7              # mask rowsum slabs on DVE; rest ACT

_CACHE: dict = {}


def _build_nc():
    from concourse import bacc, bass, bass_isa, tile
    import concourse.mybir as mybir

    f32 = mybir.dt.float32
    bf16 = mybir.dt.bfloat16
    fp8 = mybir.dt.float8e4
    i32 = mybir.dt.int32
    AX = mybir.AxisListType
    OP = mybir.AluOpType
    AF = mybir.ActivationFunctionType
    RO = bass_isa.ReduceOp

    nc = bacc.Bacc(
        "TRN2", target_bir_lowering=False, debug=False, num_devices=N_CORES
    )

    yp = nc.dram_tensor("yp", [R, WS], bf16, kind="ExternalInput")
    yt = nc.dram_tensor("yt", [R, WS], bf16, kind="ExternalInput")
    mk = nc.dram_tensor("mk", [RF, WS], fp8, kind="ExternalInput")
    meta = nc.dram_tensor("meta", [1], f32, kind="ExternalInput")
    out = nc.dram_tensor("out", [2], f32, kind="ExternalOutput")

    ypv = yp.ap().rearrange("(g u p j) w -> g p u j w", p=128, j=KJ, u=GT)
    ytv = yt.ap().rearrange("(g u p j) w -> g p u j w", p=128, j=KJ, u=GT)
    mkv = mk.ap().rearrange("(s u p j) w -> s p u j w", p=128, j=KJ, u=GT)

    with tile.TileContext(nc) as tc:
        with (
            tc.tile_pool(name="persist", bufs=1) as pp,
            tc.tile_pool(name="pp2", bufs=2) as ppool,
            tc.tile_pool(name="tp2", bufs=2) as tpool,
            tc.tile_pool(name="psp", bufs=1,
                         space=bass.MemorySpace.PSUM) as pspool,
            tc.tile_pool(name="sqp", bufs=3) as sqpool,
            tc.tile_pool(name="asc", bufs=2) as ascratch,
        ):
            from concourse.tile_rust import add_dep_helper

            # ---- setup: constants and static coordinate tiles ----
            iota_w = pp.tile([1, WS], i32, tag="iota_w")
            nc.gpsimd.iota(iota_w[:], pattern=[[1, WS]], base=0,
                           channel_multiplier=0)
            k160 = pp.tile([1, WS], f32, tag="k160")
            nc.vector.tensor_copy(out=k160[:], in_=iota_w[:])
            bmk = pp.tile([1, WS], f32, tag="bmk")
            nc.vector.tensor_scalar(out=bmk[:], in0=k160[:], scalar1=-1.0,
                                    scalar2=BIG, op0=OP.mult, op1=OP.add)
            kpb = pp.tile([1, WS], f32, tag="kpb")
            nc.vector.tensor_scalar(out=kpb[:], in0=k160[:], scalar1=BIG,
                                    scalar2=None, op0=OP.add)
            ones_f8 = pp.tile([128, 1], fp8, tag="ones_f8")
            nc.gpsimd.memset(ones_f8[:], 1.0)

            meta_s = pp.tile([1, 1], f32, tag="meta_s")
            nc.gpsimd.dma_start(
                out=meta_s[:], in_=meta.ap().rearrange("(p x) -> p x", p=1))
            meta_b = pp.tile([128, 1], f32, tag="meta_b")
            nc.gpsimd.partition_broadcast(meta_b[:], meta_s[:], channels=128)

            def coord_tiles(ncols, nt_pat, tagp):
                # r = 4p + 512t + j over (t,j); d = r//160, h = r%160
                io = pp.tile([128, ncols], i32, tag=f"io_{tagp}")
                nc.gpsimd.iota(io[:].rearrange("p (t j) -> p t j", j=KJ),
                               pattern=[[512, nt_pat], [1, KJ]], base=0,
                               channel_multiplier=4)
                rf_ = pp.tile([128, ncols], f32, tag=f"rf_{tagp}")
                nc.vector.tensor_copy(out=rf_[:], in_=io[:])
                x = pp.tile([128, ncols], f32, tag=f"x_{tagp}")
                nc.vector.tensor_scalar(out=x[:], in0=rf_[:],
                                        scalar1=1.0 / 160.0, scalar2=None,
                                        op0=OP.mult)
                di = pp.tile([128, ncols], i32, tag=f"di_{tagp}")
                nc.vector.tensor_copy(out=di[:], in_=x[:])
                df = pp.tile([128, ncols], f32, tag=f"df_{tagp}")
                nc.vector.tensor_copy(out=df[:], in_=di[:])
                co = pp.tile([128, ncols], f32, tag=f"co_{tagp}")
                nc.vector.tensor_tensor(out=co[:], in0=df[:], in1=x[:],
                                        op=OP.is_gt)
                dl = pp.tile([128, ncols], f32, tag=f"dl_{tagp}")
                nc.vector.tensor_tensor(out=dl[:], in0=df[:], in1=co[:],
                                        op=OP.subtract)
                hneg = pp.tile([128, ncols], f32, tag=f"hn_{tagp}")
                nc.vector.tensor_scalar(out=hneg[:], in0=dl[:],
                                        scalar1=-160.0, scalar2=None,
                                        op0=OP.mult)
                hl = pp.tile([128, ncols], f32, tag=f"hl_{tagp}")
                nc.vector.tensor_tensor(out=hl[:], in0=rf_[:], in1=hneg[:],
                                        op=OP.add)
                return dl, hl

            # own rows: for the in_dh weights (d needs the meta offset)
            d_own, h_own = coord_tiles(NT * KJ, NT, "own")
            d_gpc = pp.tile([128, NT * KJ], f32, tag="d_gpc")
            nc.vector.tensor_scalar(out=d_gpc[:], in0=d_own[:],
                                    scalar1=meta_b[:, 0:1], scalar2=None,
                                    op0=OP.add)
            # full-item rows: for the bbox extrema (global d, no meta)
            d_ful, h_ful = coord_tiles(RF // 128, RF // (128 * KJ), "ful")
            coefs = []
            for k, (base, sgn) in enumerate(((d_ful, -1.0), (d_ful, 1.0),
                                             (h_ful, -1.0), (h_ful, 1.0))):
                cf = pp.tile([128, RF // 128], f32, tag=f"cf_{k}")
                nc.vector.tensor_scalar(out=cf[:], in0=base[:], scalar1=sgn,
                                        scalar2=BIG, op0=OP.mult, op1=OP.add)
                coefs.append(cf)

            # ---------------- phase 1: full-mask projections ------------
            mkA = pp.tile([128, 5 * GF], fp8, tag="mkA")  # slabs 0,2,4,6,8
            mkB = pp.tile([128, 5 * GF], fp8, tag="mkB")  # slabs 1,3,5,7,9
            acc_r = pp.tile([128, RF // 128], f32, tag="acc_r")
            colps = pspool.tile([1, 2 * WS], f32, tag="colps")

            slab_ap = []
            mask_sync_last = None
            mask_scal_last = None
            for s in range(NS):
                dst = (mkA if s % 2 == 0 else mkB)
                dsl = dst[:, (s // 2) * GF : (s // 2 + 1) * GF]
                dma = (nc.sync if s % 2 == 0 else nc.scalar).dma_start(
                    out=dsl.rearrange("p (u j w) -> p u j w", u=GT, j=KJ),
                    in_=mkv[s])
                if s % 2 == 0:
                    mask_sync_last = dma
                else:
                    mask_scal_last = dma
                slab_ap.append(dsl)

            for s in range(NS):
                for c in range(GF // (2 * WS)):
                    nc.tensor.matmul(
                        colps[:], ones_f8[:],
                        slab_ap[s][:, c * 2 * WS : (c + 1) * 2 * WS],
                        start=(s == 0 and c == 0),
                        stop=(s == NS - 1 and c == GF // (2 * WS) - 1))
            # ACT takes rowsum slabs 7-9 (fills its idle mask window);
            # DVE slabs 0-6 are issued interleaved with the bulk loop below
            with nc.allow_low_precision("0/1 mask sums are exact"):
                for s in range(N_DVE_SLABS, NS):
                    for v in range(GV):
                        scv = ascratch.tile([128, WS], bf16, tag="scv")
                        nc.scalar.activation(
                            out=scv[:],
                            in_=slab_ap[s][:, v * WS : (v + 1) * WS],
                            func=AF.Copy,
                            accum_out=acc_r[:, s * GV + v : s * GV + v + 1])

            # ---------------- bbox extrema ----------------
            anyw2 = pp.tile([1, 2 * WS], f32, tag="anyw2")
            nc.scalar.activation(out=anyw2[:], in_=colps[:], func=AF.Copy)
            v_w = pp.tile([1, WS], f32, tag="v_w")
            nc.vector.tensor_tensor(out=v_w[:], in0=anyw2[:, 0:WS],
                                    in1=anyw2[:, WS : 2 * WS], op=OP.add)
            gt_w = pp.tile([1, WS], f32, tag="gt_w")
            nc.vector.tensor_scalar(out=gt_w[:], in0=v_w[:], scalar1=1.0,
                                    scalar2=None, op0=OP.min)
            ta_w = pp.tile([1, WS], f32, tag="ta_w")
            nc.vector.tensor_tensor(out=ta_w[:], in0=gt_w[:], in1=bmk[:],
                                    op=OP.mult)
            ra_w = pp.tile([1, 1], f32, tag="ra_w")
            nc.vector.tensor_reduce(out=ra_w[:], in_=ta_w[:], axis=AX.X,
                                    op=OP.max)
            tb_w = pp.tile([1, WS], f32, tag="tb_w")
            nc.vector.tensor_tensor(out=tb_w[:], in0=gt_w[:], in1=kpb[:],
                                    op=OP.mult)
            rb_w = pp.tile([1, 1], f32, tag="rb_w")
            nc.vector.tensor_reduce(out=rb_w[:], in_=tb_w[:], axis=AX.X,
                                    op=OP.max)
            hf_sum = pp.tile([1, 1], f32, tag="hf_sum")
            nc.vector.tensor_reduce(out=hf_sum[:], in_=gt_w[:], axis=AX.X,
                                    op=OP.add)
            hf_loc = pp.tile([1, 1], f32, tag="hf_loc")
            nc.vector.tensor_scalar(out=hf_loc[:], in0=hf_sum[:], scalar1=1.0,
                                    scalar2=None, op0=OP.min)

            slw = pp.tile([1, 2], f32, tag="slw")  # [-mn_w, mx_w]
            nc.vector.tensor_scalar(out=slw[:, 0:1], in0=ra_w[:],
                                    scalar1=-BIG, scalar2=None, op0=OP.add)
            nc.vector.tensor_scalar(out=slw[:, 1:2], in0=rb_w[:],
                                    scalar1=-BIG, scalar2=None, op0=OP.add)

            # ---------------- box bounds (compare form) ----------------
            def bounds(mn_neg, mx, tagp):
                # inputs: mn_neg = -mn (exact), mx; returns lo-1, hi-1
                mn = pp.tile([1, 1], f32, tag=f"mn_{tagp}")
                nc.vector.tensor_scalar(out=mn[:], in0=mn_neg, scalar1=-1.0,
                                        scalar2=None, op0=OP.mult)
                c2 = pp.tile([1, 1], f32, tag=f"c2_{tagp}")
                nc.vector.tensor_tensor(out=c2[:], in0=mn[:], in1=mx,
                                        op=OP.add)
                cC = pp.tile([1, 1], f32, tag=f"cC_{tagp}")
                nc.vector.tensor_scalar(out=cC[:], in0=c2[:], scalar1=0.5,
                                        scalar2=None, op0=OP.mult)
                em = pp.tile([1, 1], f32, tag=f"em_{tagp}")
                nc.vector.tensor_tensor(out=em[:], in0=mx, in1=mn[:],
                                        op=OP.subtract)
                nc.vector.tensor_scalar(out=em[:], in0=em[:], scalar1=1.0,
                                        scalar2=0.5, op0=OP.add, op1=OP.mult)
                eE = pp.tile([1, 1], f32, tag=f"eE_{tagp}")
                nc.vector.tensor_scalar(out=eE[:], in0=em[:], scalar1=EXPAND,
                                        scalar2=None, op0=OP.mult)
                lo = pp.tile([1, 1], f32, tag=f"lo_{tagp}")
                nc.vector.tensor_tensor(out=lo[:], in0=cC[:], in1=eE[:],
                                        op=OP.subtract)
                nc.vector.tensor_scalar(out=lo[:], in0=lo[:], scalar1=-1.0,
                                        scalar2=None, op0=OP.add)
                hi = pp.tile([1, 1], f32, tag=f"hi_{tagp}")
                nc.vector.tensor_tensor(out=hi[:], in0=cC[:], in1=eE[:],
                                        op=OP.add)
                nc.vector.tensor_scalar(out=hi[:], in0=hi[:], scalar1=-1.0,
                                        scalar2=float(WS - 2), op0=OP.add,
                                        op1=OP.min)
                return lo, hi


            lo_w, hi_w = bounds(slw[:, 0:1], slw[:, 1:2], "w")
            # w01: 0/1 weight row over w, with has_fg folded in
            in_w = pp.tile([1, WS], f32, tag="in_w")
            wk0 = pp.tile([1, WS], f32, tag="wk0")
            nc.vector.tensor_scalar(out=in_w[:], in0=k160[:],
                                    scalar1=lo_w[:], scalar2=None,
                                    op0=OP.is_gt)
            nc.vector.tensor_scalar(out=wk0[:], in0=k160[:],
                                    scalar1=hi_w[:], scalar2=None,
                                    op0=OP.is_le)
            nc.vector.tensor_tensor(out=in_w[:], in0=in_w[:], in1=wk0[:],
                                    op=OP.mult)
            nc.vector.tensor_scalar(out=in_w[:], in0=in_w[:],
                                    scalar1=hf_loc[:], scalar2=None,
                                    op0=OP.mult)
            in_w_bf = pp.tile([1, WS], bf16, tag="in_w_bf")
            with nc.allow_low_precision("0/1 weights exact in bf16"):
                nc.vector.tensor_copy(out=in_w_bf[:], in_=in_w[:])
            w01b = pp.tile([128, WS], bf16, tag="w01b")
            nc.gpsimd.partition_broadcast(w01b[:], in_w_bf[:], channels=128)
            w01rep = pp.tile([128, GF], bf16, tag="w01rep")
            w01_last = None
            with nc.allow_low_precision("0/1 weights exact in bf16"):
                for v in range(GV):
                    w01_last = nc.vector.tensor_copy(
                        out=w01rep[:, v * WS : (v + 1) * WS], in_=w01b[:])

            # ---------------- phase 2: weighted MSE sums ----------------
            lp = nc.allow_low_precision("bf16 stream; f32 accumulation")
            lp.__enter__()
            acc_tot = pp.tile([128, NG], f32, tag="acc_tot")
            wrow = pp.tile([128, NT * KJ], f32, tag="wrow")
            def dve_rowsum(s):
                with nc.allow_low_precision("0/1 mask sums are exact"):
                    nc.vector.tensor_reduce(
                        out=acc_r[:, s * GV : (s + 1) * GV],
                        in_=slab_ap[s].rearrange("p (v w) -> p v w", v=GV),
                        axis=AX.X, op=OP.add)

            dve_rowsum(0)
            dve_rowsum(1)
            for g in range(NG):
                for s2 in range(2 + g, min(2 + g + 1, N_DVE_SLABS)):
                    dve_rowsum(s2)
                p_g = ppool.tile([128, GF], bf16, tag="p_g")
                yp_dma = nc.sync.dma_start(
                    out=p_g[:].rearrange("p (u j w) -> p u j w", u=GT, j=KJ),
                    in_=ypv[g])
                t_g = tpool.tile([128, GF], bf16, tag="t_g")
                yt_dma = nc.scalar.dma_start(
                    out=t_g[:].rearrange("p (u j w) -> p u j w", u=GT, j=KJ),
                    in_=ytv[g])
                add_dep_helper(yp_dma.ins, mask_sync_last.ins, sync=False,
                               reason="mask first on sync queue")
                add_dep_helper(yp_dma.ins, mask_scal_last.ins, sync=True,
                               reason="mask first (cross queue)")
                add_dep_helper(yt_dma.ins, mask_scal_last.ins, sync=False,
                               reason="mask first on scalar queue")
                add_dep_helper(yt_dma.ins, mask_sync_last.ins, sync=True,
                               reason="mask first (cross queue)")
                sub_i = nc.vector.tensor_tensor(out=p_g[:], in0=p_g[:],
                                                in1=t_g[:], op=OP.subtract)
                sq_g = sqpool.tile([128, GF], bf16, tag="sq_g")
                nc.scalar.activation(
                    out=sq_g[:], in_=p_g[:], func=AF.Square,
                    accum_out=acc_tot[:, g : g + 1])
                nc.vector.tensor_tensor(out=sq_g[:], in0=sq_g[:],
                                        in1=w01rep[:], op=OP.mult)
                nc.vector.tensor_reduce(
                    out=wrow[:, g * GV : (g + 1) * GV],
                    in_=sq_g[:].rearrange("p (v w) -> p v w", v=GV),
                    axis=AX.X, op=OP.add)
            lp.__exit__(None, None, None)

            gt_r = pp.tile([128, RF // 128], f32, tag="gt_r")
            nc.vector.tensor_scalar(out=gt_r[:], in0=acc_r[:], scalar1=1.0,
                                    scalar2=None, op0=OP.min)
            scr = pp.tile([128, RF // 128], f32, tag="scr")
            p4 = pp.tile([128, 4], f32, tag="p4")
            for k in range(4):
                nc.vector.tensor_tensor(out=scr[:], in0=gt_r[:],
                                        in1=coefs[k][:], op=OP.mult)
                nc.vector.tensor_reduce(out=p4[:, k : k + 1], in_=scr[:],
                                        axis=AX.X, op=OP.max)
            p4r = pp.tile([128, 4], f32, tag="p4r")
            nc.gpsimd.partition_all_reduce(p4r[:], p4[:], channels=128,
                                           reduce_op=RO.max)
            sl4 = pp.tile([1, 4], f32, tag="sl4")  # [-mn_d, mx_d, -mn_h, mx_h]
            nc.vector.tensor_scalar(out=sl4[:], in0=p4r[0:1, :],
                                    scalar1=-BIG, scalar2=None, op0=OP.add)
            lo_d, hi_d = bounds(sl4[:, 0:1], sl4[:, 1:2], "d")
            lo_h, hi_h = bounds(sl4[:, 2:3], sl4[:, 3:4], "h")
            # in_dh: per-own-row 0/1 weight from d/h bounds
            b4 = pp.tile([1, 4], f32, tag="b4")
            for k, srcb in enumerate((lo_d, hi_d, lo_h, hi_h)):
                nc.vector.tensor_copy(out=b4[:, k : k + 1], in_=srcb[:])
            b4b = pp.tile([128, 4], f32, tag="b4b")
            nc.gpsimd.partition_broadcast(b4b[:], b4[:], channels=128)
            in_dh = pp.tile([128, NT * KJ], f32, tag="in_dh")
            wk1 = pp.tile([128, NT * KJ], f32, tag="wk1")
            nc.vector.tensor_scalar(out=in_dh[:], in0=d_gpc[:],
                                    scalar1=b4b[:, 0:1], scalar2=None,
                                    op0=OP.is_gt)
            nc.vector.tensor_scalar(out=wk1[:], in0=d_gpc[:],
                                    scalar1=b4b[:, 1:2], scalar2=None,
                                    op0=OP.is_le)
            nc.vector.tensor_tensor(out=in_dh[:], in0=in_dh[:], in1=wk1[:],
                                    op=OP.mult)
            nc.vector.tensor_scalar(out=wk1[:], in0=h_own[:],
                                    scalar1=b4b[:, 2:3], scalar2=None,
                                    op0=OP.is_gt)
            nc.vector.tensor_tensor(out=in_dh[:], in0=in_dh[:], in1=wk1[:],
                                    op=OP.mult)
            nc.vector.tensor_scalar(out=wk1[:], in0=h_own[:],
                                    scalar1=b4b[:, 3:4], scalar2=None,
                                    op0=OP.is_le)
            nc.vector.tensor_tensor(out=in_dh[:], in0=in_dh[:], in1=wk1[:],
                                    op=OP.mult)

            # ---------------- final reductions ----------------
            tot_col = pp.tile([128, 1], f32, tag="tot_col")
            nc.vector.tensor_reduce(out=tot_col[:], in_=acc_tot[:],
                                    axis=AX.X, op=OP.add)
            junk_a = pp.tile([128, NT * KJ], f32, tag="junk_a")
            nc.vector.tensor_tensor(out=junk_a[:], in0=wrow[:],
                                    in1=in_dh[:], op=OP.mult)
            box_col = pp.tile([128, 1], f32, tag="box_col")
            nc.vector.tensor_reduce(out=box_col[:], in_=junk_a[:], axis=AX.X,
                                    op=OP.add)
            tot_r = pp.tile([128, 1], f32, tag="tot_r")
            nc.gpsimd.partition_all_reduce(tot_r[:], tot_col[:], channels=128,
                                           reduce_op=RO.add)
            box_r = pp.tile([128, 1], f32, tag="box_r")
            nc.gpsimd.partition_all_reduce(box_r[:], box_col[:], channels=128,
                                           reduce_op=RO.add)
            res2 = pp.tile([1, 2], f32, tag="res2")
            nc.vector.tensor_copy(out=res2[:, 0:1], in_=tot_r[0:1, :])
            nc.vector.tensor_copy(out=res2[:, 1:2], in_=box_r[0:1, :])
            nc.gpsimd.dma_start(
                out=out.ap().rearrange("(p x) -> p x", p=1), in_=res2[:])

    nc.compile()
    return nc


def get_nc():
    if "nc" not in _CACHE:
        _CACHE["nc"] = _build_nc()
    return _CACHE["nc"]


def make_in_maps(y_pred, y_true, mask):
    import ml_dtypes

    y_pred = np.asarray(y_pred, dtype=np.float32).reshape(B, DS, HS, WS)
    y_true = np.asarray(y_true, dtype=np.float32).reshape(B, DS, HS, WS)
    mask = np.asarray(mask, dtype=np.int32).reshape(B, DS, HS, WS)
    y_pred = y_pred.astype(ml_dtypes.bfloat16)
    y_true = y_true.astype(ml_dtypes.bfloat16)
    mask_f8 = mask.astype(ml_dtypes.float8_e4m3)  # 0/1 values: exact
    in_maps = []
    for c in range(N_CORES):
        b, half = c // 2, c % 2
        sl = slice(half * HALF_D, (half + 1) * HALF_D)
        in_maps.append({
            "yp": np.ascontiguousarray(y_pred[b, sl]).reshape(R, WS),
            "yt": np.ascontiguousarray(y_true[b, sl]).reshape(R, WS),
            "mk": np.ascontiguousarray(mask_f8[b]).reshape(RF, WS),
            "meta": np.array([half * HALF_D], dtype=np.float32),
        })
    return in_maps


def combine(results):
    tot = 0.0
    box = 0.0
    for r in results:
        o = np.asarray(r["out"], dtype=np.float64).reshape(-1)
        tot += o[0]
        box += o[1]
    loss = (W_OUT2 * tot + (1.0 - W_OUT2) * box) / float(B * DS * HS * WS)
    return np.array(loss, dtype=np.float32)


def kernel(y_pred, y_true, mask):
    from concourse.bass_utils import run_bass_kernel_spmd

    nc = get_nc()
    in_maps = make_in_maps(y_pred, y_true, mask)
    trace = bool(int(os.environ.get("BASS_KERNEL_TRACE", "0")))
    kwargs = {}
    if trace:
        kwargs = dict(trace=True, trace_cores=[0])
    res = run_bass_kernel_spmd(
        nc, in_maps, core_ids=list(range(N_CORES)), **kwargs
    )
    _CACHE["last_results"] = res
    return combine(res.results)
